# revision 1
# baseline (speedup 1.0000x reference)
"""Trainium2 Bass kernel for nn_FC_LSTM (FC-LSTM encoder-decoder).

Strategy:
  - Data-parallel over batch: 256 samples -> 8 cores x 32 samples.
  - Feature-major layout on chip: activations stored [feature(part), sample(free)],
    weights host-transposed to [in_feat, out_feat] so every matmul is
    out[feat_out, samples] = W_T.T @ act with contraction on partitions.
  - Encoder FC stack (4096->1024->256) batched over all 20 timesteps (640
    samples per core). en1's input-side gate matmul (Wih @ z_t) is also
    precomputed densely for all t.
  - LSTM recurrence keeps gates packed in one PSUM bank [128, 8*32]: psum col
    m*32+s = gate-feature 128m+p of sample s. Gate order host-permuted to
    [i, f, o, g] so sigmoid covers cols 0..191 and tanh 192..255 in one ACT
    instruction each. Cell biases are injected with a single K=128 matmul
    against a constant indicator matrix.
  - Decoder FC stack (256->1024->4096) batched over all future steps.
  - bf16 matmul operands (fp32 PSUM accumulation), fp32 cell state.
  - The FC phases are emitted as generators whose matmul blocks interleave
    into the recurrence as PE gap fillers; cell emission is software-
    pipelined (en2(t), en1(t+1), en3(t)) so elementwise latency hides under
    other cells' matmuls. DMAs are emitted in first-use order.
  - measured ~160-230us/iteration steady-state on 8 axon-tunneled trn2
    cores (cost-model sim of the single-shot program: ~255us).
"""

import time
from contextlib import ExitStack

import ml_dtypes
import numpy as np

import concourse.bass as bass
import concourse.mybir as mybir
import concourse.tile as tile

BF16NP = ml_dtypes.bfloat16
AF = mybir.ActivationFunctionType
DT = mybir.dt

S = 20          # encoder sequence length
B = 256         # global batch
NCORES = 8
BL = B // NCORES  # 32 samples per core
H = 256         # LSTM hidden
G = 4 * H       # 1024 gate features
D = 4096        # input feature dim (64*64)
HID = 1024      # FC hidden
SB = S * BL     # 640 encoder samples per core

VERBOSE = True


def _log(*a):
    if VERBOSE:
        print("[kernel]", *a, flush=True)


# ---------------------------------------------------------------------------
# Workaround: walrus CoreV3 setupSyncWait allows only 1 sync wait on the
# TileContext exit Drain. Split its waits across multiple drain instructions.
# ---------------------------------------------------------------------------
def _patched_drain_and_barrier(self, tick_clock, wait_clock):
    nc = self.nc
    drain_inst = nc.sync.drain()
    wait_clock.add_sem_waits(
        drain_inst.ins, tile.ScopedClock({None: tick_clock.global_clock})
    )
    inst = drain_inst.ins
    si = inst.sync_info
    waits = list(si.on_wait) if si is not None and si.on_wait else []
    MAXW = 1
    if len(waits) > MAXW:
        si.on_wait = waits[:MAXW]
        for i in range(MAXW, len(waits), MAXW):
            d2 = nc.sync.drain()
            i2 = d2.ins
            si2 = i2.sync_info
            if si2 is None:
                i2.sync_info = type(si)(on_wait=waits[i : i + MAXW], on_update=[])
            else:
                si2.on_wait = list(si2.on_wait or []) + waits[i : i + MAXW]

    nc.all_engine_barrier()
    assert self.sems is not None
    popped = nc._tile_sem_poison_stack.pop()
    assert popped is self._sem_poison
    nc.clear_and_free_semaphores(list(self.sems.allocated().values()))
    nc.all_engine_barrier()


tile.TileContext._drain_and_barrier = _patched_drain_and_barrier


def _split_sync_waits(nc, limit=1):
    """walrus setupSyncWait rejects >2 sem waits per instruction: move excess
    waits onto same-engine NoOps spliced just before the instruction."""
    ctr = [0]
    SyncInfo = None
    for f in nc.m.functions:
        for bb in f.blocks:
            out = []
            for inst in bb.instructions:
                si = inst.sync_info
                waits = list(si.on_wait) if si is not None and si.on_wait else []
                if len(waits) > limit:
                    if SyncInfo is None:
                        SyncInfo = type(si)
                    extras = waits[: len(waits) - limit]
                    si.on_wait = waits[len(waits) - limit:]
                    for i in range(0, len(extras), limit):
                        ctr[0] += 1
                        nop = mybir.InstNoOp(name=f"ws_{ctr[0]}", ins=[], outs=[])
                        nop.engine = inst.engine
                        nop.sync_info = SyncInfo(
                            on_wait=extras[i : i + limit], on_update=[]
                        )
                        out.append(nop)
                out.append(inst)
            bb.instructions[:] = out
    return ctr[0]


# ---------------------------------------------------------------------------
# Program builder
# ---------------------------------------------------------------------------
CELLS = ["en1", "en2", "en3", "de1", "de2", "de3"]


def build_program(F: int, nrep: int = 1) -> bass.Bass:
    FB = F * BL  # decoder samples per core
    assert FB <= 512 and FB % 2 == 0
    nc = bass.Bass()

    # --- DRAM tensors ---
    xT = nc.dram_tensor("xT", [D, SB], DT.bfloat16, kind="ExternalInput")
    w1T = nc.dram_tensor("w1T", [D, HID], DT.bfloat16, kind="ExternalInput")
    b1v = nc.dram_tensor("b1v", [128, HID // 128], DT.float32, kind="ExternalInput")
    w2T = nc.dram_tensor("w2T", [HID, H], DT.bfloat16, kind="ExternalInput")
    b2v = nc.dram_tensor("b2v", [128, H // 128], DT.float32, kind="ExternalInput")
    cellT = {}
    for nm in CELLS:
        ins = {}
        if nm != "de1":
            ins["wih"] = nc.dram_tensor(f"{nm}_wihT", [H, G], DT.bfloat16,
                                        kind="ExternalInput")
        ins["whh"] = nc.dram_tensor(f"{nm}_whhT", [H, G], DT.bfloat16,
                                    kind="ExternalInput")
        if nm == "en1":
            ins["bv"] = nc.dram_tensor("en1_bv", [128, G // 128], DT.float32,
                                       kind="ExternalInput")
        else:
            ins["bB"] = nc.dram_tensor(f"{nm}_bB", [128, 128], DT.bfloat16,
                                       kind="ExternalInput")
        cellT[nm] = ins
    Em = nc.dram_tensor("Em", [128, 256], DT.bfloat16, kind="ExternalInput")
    wd1T = nc.dram_tensor("wd1T", [H, HID], DT.bfloat16, kind="ExternalInput")
    bd1v = nc.dram_tensor("bd1v", [128, HID // 128], DT.float32, kind="ExternalInput")
    wd2T = nc.dram_tensor("wd2T", [HID, D], DT.bfloat16, kind="ExternalInput")
    bd2v = nc.dram_tensor("bd2v", [128, D // 128], DT.float32, kind="ExternalInput")
    yT = nc.dram_tensor("yT", [D, FB], DT.float32, kind="ExternalOutput")

    NCHA = 4                      # phase A chunks (5 encoder steps each)
    CHA = SB // NCHA              # 160 samples
    SPC = S // NCHA               # steps per chunk
    NCHD = 2                      # phase D chunks
    CHD = FB // NCHD

    with tile.TileContext(nc) as tc:
     for rep in range(nrep):
      with ExitStack() as ctx:
        const = ctx.enter_context(tc.tile_pool(name="const", bufs=1))
        gi1p = ctx.enter_context(tc.tile_pool(name="gi1p", bufs=1))
        state = ctx.enter_context(tc.tile_pool(name="state", bufs=3))
        gates = ctx.enter_context(tc.tile_pool(name="gates", bufs=4))
        outp = ctx.enter_context(tc.tile_pool(name="outp", bufs=4))
        psum = ctx.enter_context(tc.tile_pool(name="psum", bufs=8, space="PSUM"))

        uid = [0]

        def PS():
            uid[0] += 1
            return psum.tile([128, 512], DT.float32, tag="ps",
                             name=f"ps{uid[0]}")

        def dma_in(pool, dram, kshape, tag):
            """Load [K, M] dram weight into [128, K//128, M] sbuf tile."""
            k, m = kshape
            t = pool.tile([128, k // 128, m], dram.dtype, tag=tag)
            nc.sync.dma_start(t[:], dram.rearrange("(o p) m -> p o m", p=128))
            return t

        def dma_in2(pool, dram, tag):
            t = pool.tile(list(dram.shape), dram.dtype, tag=tag)
            nc.sync.dma_start(t[:], dram[:])
            return t

        def gi_ap(t):
            c, o = divmod(t, SPC)
            return gi1[c][:, :, o * BL:(o + 1) * BL]

        # ========== Phase A generator: per-k-tile weight/x DMAs, chunked ===
        pA_ctx = ExitStack()
        pA = pA_ctx.enter_context(tc.tile_pool(name="phaseA", bufs=1))

        # DMA emission ordered by first use: w1 + x chunk 0 feed the FC chain
        # immediately; encoder cell weights arrive next; remaining x chunks
        # stream during the early recurrence; decoder weights last.
        x_kc = [[None] * NCHA for _ in range(D // 128)]
        w1_k = []
        for k in range(D // 128):
            wk = pA.tile([128, HID], DT.bfloat16, tag=f"w1_{k}", name=f"w1_{k}")
            nc.sync.dma_start(wk[:], w1T[k * 128:(k + 1) * 128, :])
            w1_k.append(wk)
            xk = pA.tile([128, CHA], DT.bfloat16, tag=f"x{k}_0", name=f"x{k}_0")
            nc.sync.dma_start(xk[:], xT[k * 128:(k + 1) * 128, 0:CHA])
            x_kc[k][0] = xk

        # encoder-phase constants
        b1_sb = dma_in2(const, b1v, "b1v")
        b2_sb = dma_in2(const, b2v, "b2v")
        w2_sb = dma_in(const, w2T, (HID, H), "w2")
        cell_sb = {}
        for nm in ["en1", "en2", "en3"]:
            e = {}
            e["wih"] = dma_in(const, cellT[nm]["wih"], (H, G), f"{nm}_wih")
            e["whh"] = dma_in(const, cellT[nm]["whh"], (H, G), f"{nm}_whh")
            if nm == "en1":
                e["bv"] = dma_in2(const, cellT[nm]["bv"], "en1_bv")
            else:
                e["bB"] = dma_in2(const, cellT[nm]["bB"], f"{nm}_bB")
            cell_sb[nm] = e
        E_sb = dma_in2(const, Em, "Em")

        # remaining x chunks
        for c in range(1, NCHA):
            for k in range(D // 128):
                xk = pA.tile([128, CHA], DT.bfloat16, tag=f"x{k}_{c}",
                             name=f"x{k}_{c}")
                nc.sync.dma_start(
                    xk[:], xT[k * 128:(k + 1) * 128, c * CHA:(c + 1) * CHA])
                x_kc[k][c] = xk

        # decoder-phase constants
        for nm in ["de1", "de2", "de3"]:
            e = {}
            if "wih" in cellT[nm]:
                e["wih"] = dma_in(const, cellT[nm]["wih"], (H, G), f"{nm}_wih")
            e["whh"] = dma_in(const, cellT[nm]["whh"], (H, G), f"{nm}_whh")
            e["bB"] = dma_in2(const, cellT[nm]["bB"], f"{nm}_bB")
            cell_sb[nm] = e
        wd1_sb = dma_in(const, wd1T, (H, HID), "wd1")
        bd1_sb = dma_in2(const, bd1v, "bd1v")
        bd2_sb = dma_in2(const, bd2v, "bd2v")
        zh = const.tile([128, 2 * BL], DT.bfloat16, tag="zh")
        zc = const.tile([128, 2 * BL], DT.float32, tag="zc")
        nc.vector.memset(zh[:], 0.0)
        nc.vector.memset(zc[:], 0.0)
        h3all = [const.tile([128, H // 128, CHD], DT.bfloat16, tag=f"h3all{c}",
                            name=f"h3all{c}")
                 for c in range(NCHD)]
        gi1 = [gi1p.tile([128, G // 128, CHA], DT.float32, tag=f"gi1_{c}",
                         name=f"gi1_{c}")
               for c in range(NCHA)]


        def phaseA_gen():
            """Emits all of phase A; yields chunks_done after each MM block."""
            for c in range(NCHA):
                n0 = c * CHA
                z1c = pA.tile([128, HID // 128, CHA], DT.bfloat16,
                              tag=f"z1_{c}", name=f"z1_{c}")
                zc_ = pA.tile([128, H // 128, CHA], DT.bfloat16,
                              tag=f"z_{c}", name=f"z_{c}")
                if c == 0:
                    # chunk 0 runs while x/w1 DMAs land: k-outer so each
                    # arriving k-tile is consumed immediately
                    for half in range(2):
                        ms = range(half * 4, half * 4 + 4)
                        pss = [PS()[:, :CHA] for _ in ms]
                        for k in range(D // 128):
                            for mi, m in enumerate(ms):
                                nc.tensor.matmul(
                                    pss[mi], w1_k[k][:, m * 128:(m + 1) * 128],
                                    x_kc[k][0][:],
                                    start=(k == 0), stop=(k == D // 128 - 1),
                                )
                            if k % 8 == 7:
                                yield c
                        for mi, m in enumerate(ms):
                            nc.vector.tensor_scalar(
                                z1c[:, m, :], pss[mi], b1_sb[:, m:m + 1], 0.0,
                                mybir.AluOpType.add, mybir.AluOpType.max)
                else:
                    for m in range(HID // 128):
                        ps = PS()[:, :CHA]
                        for k0 in range(0, D // 128, 8):
                            for k in range(k0, k0 + 8):
                                nc.tensor.matmul(
                                    ps, w1_k[k][:, m * 128:(m + 1) * 128],
                                    x_kc[k][c][:],
                                    start=(k == 0), stop=(k == D // 128 - 1),
                                )
                            yield c
                        nc.vector.tensor_scalar(
                            z1c[:, m, :], ps, b1_sb[:, m:m + 1], 0.0,
                            mybir.AluOpType.add, mybir.AluOpType.max)
                for m in range(H // 128):
                    ps = PS()[:, :CHA]
                    for k in range(HID // 128):
                        nc.tensor.matmul(
                            ps, w2_sb[:, k, m * 128:(m + 1) * 128],
                            z1c[:, k, :],
                            start=(k == 0), stop=(k == HID // 128 - 1),
                        )
                    nc.vector.tensor_scalar(
                        zc_[:, m, :], ps, b2_sb[:, m:m + 1], 0.0,
                        mybir.AluOpType.add, mybir.AluOpType.max)
                    yield c
                for m in range(G // 128):
                    ps = PS()[:, :CHA]
                    for k in range(H // 128):
                        nc.tensor.matmul(
                            ps, cell_sb["en1"]["wih"][:, k, m * 128:(m + 1) * 128],
                            zc_[:, k, :],
                            start=(k == 0), stop=(k == H // 128 - 1),
                        )
                    nc.vector.tensor_scalar_add(
                        gi1[c][:, m, :], ps, cell_sb["en1"]["bv"][:, m:m + 1])
                    if m % 2 == 1:
                        yield c + (m == G // 128 - 1)
            while True:
                yield NCHA + 1

        genA = phaseA_gen()
        a_done = [0]

        def fillA(n=1):
            for _ in range(n):
                a_done[0] = max(a_done[0], next(genA))

        def needA(chunks):
            while a_done[0] < chunks + 1:
                fillA()

        # ========== LSTM cell =============================================
        def lstm_cell(nm, x_in, h_prev, c_prev, gi, htag, ctag):
            e = cell_sb[nm]
            ps = PS()[:, :256]
            groups = []
            if gi is None:
                nc.tensor.matmul(ps, e["bB"][:], E_sb[:], start=True, stop=False)
                started = True
            else:
                started = False
            if x_in is not None:
                groups.append((e["wih"], x_in))
            groups.append((e["whh"], h_prev))
            ng = len(groups)
            for gidx, (w_sb, rhs) in enumerate(groups):
                for k in range(2):
                    for m in range(8):
                        nc.tensor.matmul(
                            ps[:, m * 32:(m + 1) * 32],
                            w_sb[:, k, m * 128:(m + 1) * 128],
                            rhs[:, k * 32:(k + 1) * 32],
                            start=(not started and gidx == 0 and k == 0),
                            stop=(gidx == ng - 1 and k == 1),
                            skip_group_check=True,
                        )
            if gi is not None:
                pv = ps.rearrange("p (m s) -> p m s", s=32)
                nc.vector.tensor_add(pv, pv, gi)
            g = gates.tile([128, 256], DT.float32, tag="g", name=f"g{uid[0]}")
            nc.scalar.activation(g[:, 0:192], ps[:, 0:192], AF.Sigmoid)
            nc.scalar.activation(g[:, 192:256], ps[:, 192:256], AF.Tanh)
            # packed: i: 0..63, f: 64..127, o: 128..191, g: 192..255
            t1 = gates.tile([128, 64], DT.float32, tag="t1", name=f"t1{uid[0]}")
            nc.vector.tensor_mul(t1[:], g[:, 0:64], g[:, 192:256])
            c_new = state.tile([128, 64], DT.float32, tag=ctag,
                               name=f"{ctag}{uid[0]}")
            nc.vector.tensor_mul(c_new[:], g[:, 64:128], c_prev[:])
            nc.vector.tensor_add(c_new[:], c_new[:], t1[:])
            th = gates.tile([128, 64], DT.float32, tag="th", name=f"th{uid[0]}")
            nc.scalar.activation(th[:], c_new[:], AF.Tanh)
            h_new = state.tile([128, 64], DT.bfloat16, tag=htag,
                               name=f"{htag}{uid[0]}")
            nc.vector.tensor_mul(h_new[:], g[:, 128:192], th[:])
            return h_new, c_new

        # ========== Phase D transition (callable mid-encoder) =============
        wd2_k = []
        pD_box = []

        def ensure_pD():
            if pD_box:
                return
            pA_ctx.close()  # free phase A SBUF before loading decoder weights
            pD = ctx.enter_context(tc.tile_pool(name="phaseD", bufs=1))
            pD_box.append(pD)
            for k in range(HID // 128):
                wk = pD.tile([128, D], DT.bfloat16, tag=f"wd2_{k}",
                             name=f"wd2_{k}")
                nc.sync.dma_start(wk[:], wd2T[k * 128:(k + 1) * 128, :])
                wd2_k.append(wk)

        # ========== Encoder recurrence, pipelined w/ phase A fillers ======
        needA(0)
        h1s, c1s = [None] * S, [None] * S
        h2s, c2s = [None] * S, [None] * S
        h3s, c3s = [None] * S, [None] * S
        h1s[0], c1s[0] = lstm_cell("en1", None, zh, zc, gi_ap(0), "h1", "c1")
        for t in range(S):
            fillA(2)
            h2p = h2s[t - 1] if t else zh
            c2p = c2s[t - 1] if t else zc
            h2s[t], c2s[t] = lstm_cell("en2", h1s[t], h2p, c2p, None, "h2", "c2")
            if t + 1 < S:
                fillA(2)
                needA((t + 1) // SPC)
                h1s[t + 1], c1s[t + 1] = lstm_cell(
                    "en1", None, h1s[t], c1s[t], gi_ap(t + 1), "h1", "c1")
            fillA(2)
            h3p = h3s[t - 1] if t else zh
            c3p = c3s[t - 1] if t else zc
            h3s[t], c3s[t] = lstm_cell("en3", h2s[t], h3p, c3p, None, "h3", "c3")
            if t == S - 5:
                needA(NCHA)  # drain phase A now so decoder weights can load
                ensure_pD()

        ensure_pD()
        pD = pD_box[0]

        def phaseD_gen(c):
            y1c = pD.tile([128, HID // 128, CHD], DT.bfloat16,
                          tag=f"y1_{c}", name=f"y1_{c}")
            for m in range(HID // 128):
                ps = PS()[:, :CHD]
                for k in range(H // 128):
                    nc.tensor.matmul(
                        ps, wd1_sb[:, k, m * 128:(m + 1) * 128],
                        h3all[c][:, k, :],
                        start=(k == 0), stop=(k == H // 128 - 1),
                    )
                nc.vector.tensor_scalar(
                    y1c[:, m, :], ps, bd1_sb[:, m:m + 1], 0.0,
                    mybir.AluOpType.add, mybir.AluOpType.max)
                if m % 2 == 1:
                    yield
            for m in range(D // 128):
                ps = PS()[:, :CHD]
                for k in range(HID // 128):
                    nc.tensor.matmul(
                        ps, wd2_k[k][:, m * 128:(m + 1) * 128],
                        y1c[:, k, :],
                        start=(k == 0), stop=(k == HID // 128 - 1),
                    )
                o_sb = outp.tile([128, CHD], DT.float32, tag="o",
                                 name=f"o{uid[0]}")
                nc.scalar.activation(o_sb[:], ps, AF.Tanh,
                                     bias=bd2_sb[:, m:m + 1])
                nc.sync.dma_start(
                    yT[m * 128:(m + 1) * 128, c * CHD:(c + 1) * CHD],
                    o_sb[:])
                yield

        genDs = [phaseD_gen(c) for c in range(NCHD)]
        d_ready = [0]   # decoder chunks whose h3all is complete

        def fillD(n=1):
            for _ in range(n):
                for c in range(d_ready[0]):
                    if next(genDs[c], None) is not None:
                        break

        # ========== Decoder recurrence, pipelined w/ phase D fillers ======
        d1s, f1s = [None] * F, [None] * F
        d2s, f2s = [None] * F, [None] * F
        d3s, f3s = [None] * F, [None] * F
        d1s[0], f1s[0] = lstm_cell("de1", None, h3s[S - 1], zc, None, "d1", "e1")
        for t in range(F):
            fillD(2)
            d2p = d2s[t - 1] if t else zh
            f2p = f2s[t - 1] if t else zc
            d2s[t], f2s[t] = lstm_cell("de2", d1s[t], d2p, f2p, None, "d2", "e2")
            if t + 1 < F:
                fillD(2)
                d1s[t + 1], f1s[t + 1] = lstm_cell(
                    "de1", None, d1s[t], f1s[t], None, "d1", "e1")
            fillD(2)
            d3p = d3s[t - 1] if t else zh
            f3p = f3s[t - 1] if t else zc
            d3s[t], f3s[t] = lstm_cell("de3", d2s[t], d3p, f3p, None, "d3", "e3")
            c, o = divmod(t, CHD // BL)
            nc.vector.tensor_copy(
                h3all[c][:, :, o * BL:(o + 1) * BL],
                d3s[t].rearrange("p (k s) -> p k s", s=BL),
            )
            if o == CHD // BL - 1:
                d_ready[0] = c + 1
        # drain remaining phase D work
        for gd in genDs:
            for _ in gd:
                pass

    nsplit = _split_sync_waits(nc, limit=1)
    _log(f"split {nsplit} over-limit sync waits")
    return nc

# ---------------------------------------------------------------------------
# Host-side input prep
# ---------------------------------------------------------------------------
GATE_PERM = np.concatenate([
    np.arange(0, 2 * H),          # i, f
    np.arange(3 * H, 4 * H),      # o
    np.arange(2 * H, 3 * H),      # g
])


def prep_inputs(inputs):
    f32 = np.float32
    g = {k: np.asarray(v) for k, v in inputs.items()}
    F = int(np.asarray(g["future_step"]))

    def bf(a):
        return np.ascontiguousarray(a).astype(BF16NP)

    shared = {}
    shared["w1T"] = bf(g["fc_en1_w"].T)
    shared["b1v"] = np.ascontiguousarray(g["fc_en1_b"].astype(f32).reshape(HID // 128, 128).T)
    shared["w2T"] = bf(g["fc_en2_w"].T)
    shared["b2v"] = np.ascontiguousarray(g["fc_en2_b"].astype(f32).reshape(H // 128, 128).T)
    for nm in CELLS:
        wih = g[nm + "_wih"][GATE_PERM]
        whh = g[nm + "_whh"][GATE_PERM]
        bsum = (g[nm + "_bih"] + g[nm + "_bhh"])[GATE_PERM].astype(f32)
        if nm != "de1":
            shared[nm + "_wihT"] = bf(wih.T)
        shared[nm + "_whhT"] = bf(whh.T)
        if nm == "en1":
            shared["en1_bv"] = np.ascontiguousarray(bsum.reshape(G // 128, 128).T)
        else:
            bB = np.zeros((128, 128), f32)
            bB[:G // 128, :] = bsum.reshape(G // 128, 128)
            shared[nm + "_bB"] = bB.astype(BF16NP)
    E = np.zeros((128, 256), f32)
    for j in range(8):
        E[j, j * 32:(j + 1) * 32] = 1.0
    shared["Em"] = E.astype(BF16NP)
    shared["wd1T"] = bf(g["fc_de1_w"].T)
    shared["bd1v"] = np.ascontiguousarray(g["fc_de1_b"].astype(f32).reshape(HID // 128, 128).T)
    shared["wd2T"] = bf(g["fc_de2_w"].T)
    shared["bd2v"] = np.ascontiguousarray(g["fc_de2_b"].astype(f32).reshape(D // 128, 128).T)

    x = g["x"].astype(f32).reshape(S, B, D)
    in_maps = []
    for c in range(NCORES):
        xc = x[:, c * BL:(c + 1) * BL, :].reshape(SB, D)   # row = t*BL + b
        m = dict(shared)
        m["xT"] = bf(xc.T)
        in_maps.append(m)
    return in_maps, F


# ---------------------------------------------------------------------------
# Execution via PJRT (axon), modeled on bass2jax.run_bass_via_pjrt
# ---------------------------------------------------------------------------
def run_spmd(nc, in_maps, n_timing=0):
    import jax
    from jax.experimental.shard_map import shard_map
    from jax.sharding import Mesh, NamedSharding, PartitionSpec

    from concourse import bass2jax

    bass2jax.install_neuronx_cc_hook()
    n_cores = len(in_maps)
    partition_name = nc.partition_id_tensor.name if nc.partition_id_tensor else None
    in_names, out_names, out_avals, zero_outs = [], [], [], []
    for alloc in nc.m.functions[0].allocations:
        if not isinstance(alloc, mybir.MemoryLocationSet):
            continue
        name = alloc.memorylocations[0].name
        if alloc.kind == "ExternalInput":
            if name != partition_name:
                in_names.append(name)
        elif alloc.kind == "ExternalOutput":
            out_names.append(name)
            shape = tuple(alloc.tensor_shape)
            dtype = mybir.dt.np(alloc.dtype)
            out_avals.append(jax.core.ShapedArray(shape, dtype))
            zero_outs.append(np.zeros(shape, dtype))
    n_params = len(in_names)
    all_in = in_names + out_names
    if partition_name is not None:
        all_in = all_in + [partition_name]
    all_in = tuple(all_in)

    def _bind(args):
        operands = list(args)
        if partition_name is not None:
            operands.append(bass2jax.partition_id_tensor())
        return bass2jax._bass_exec_p.bind(
            *operands,
            out_avals=tuple(out_avals),
            in_names=all_in,
            out_names=tuple(out_names),
            lowering_input_output_aliases=(),
            sim_require_finite=False,
            sim_require_nnan=False,
            nc=nc,
        )

    def _body(*args):
        return tuple(_bind(args))

    devices = jax.devices()[:n_cores]
    mesh = Mesh(np.asarray(devices), ("core",))
    pspec = PartitionSpec("core")
    in_specs = (pspec,) * (n_params + len(out_names))
    out_specs = (pspec,) * len(out_names)

    f1 = jax.jit(shard_map(_body, mesh=mesh, in_specs=in_specs,
                           out_specs=out_specs, check_rep=False))
    concat = [
        np.concatenate([np.asarray(in_maps[c][nm]) for c in range(n_cores)], axis=0)
        for nm in in_names
    ]
    concat += [np.concatenate([z] * n_cores, axis=0) for z in zero_outs]

    sharding = NamedSharding(mesh, pspec)
    t0 = time.perf_counter()
    dev_in = [jax.device_put(a, sharding) for a in concat]
    jax.block_until_ready(dev_in)
    _log(f"upload {sum(a.nbytes for a in concat)/1e6:.1f} MB in "
         f"{time.perf_counter()-t0:.2f}s")

    t0 = time.perf_counter()
    outs = jax.block_until_ready(f1(*dev_in))
    _log(f"first run (incl compile) {time.perf_counter()-t0:.1f}s")

    results = []
    np_outs = [np.asarray(o) for o in outs]
    for c in range(n_cores):
        r = {}
        for i, nm in enumerate(out_names):
            sh0 = out_avals[i].shape[0]
            r[nm] = np_outs[i][c * sh0:(c + 1) * sh0]
        results.append(r)

    wall = None
    if n_timing:
        ts = []
        for _ in range(n_timing):
            t0 = time.perf_counter()
            jax.block_until_ready(f1(*dev_in))
            ts.append(time.perf_counter() - t0)
        wall = min(ts)
        _log("wall per call ms: " + " ".join(f"{t*1e3:.2f}" for t in ts))
    return results, wall, (f1, dev_in)


def measure_hw_time(F, in_maps, nrep=9, reps=14):
    """HW exec estimate: (wall(nrep-program) - wall(1-program)) / (nrep-1),
    with the two programs timed in an interleaved loop to cancel drift.
    Host-side RPC jitter is ~1ms so this is accurate to roughly +-50us."""
    import jax

    nc1 = build_program(F, nrep=1)
    _, _, (f1, dev1) = run_spmd(nc1, in_maps)
    ncN = build_program(F, nrep=nrep)
    _, _, (fN, devN) = run_spmd(ncN, in_maps)
    t1s, tNs = [], []
    for _ in range(reps):
        t0 = time.perf_counter()
        jax.block_until_ready(f1(*dev1))
        t1s.append(time.perf_counter() - t0)
        t0 = time.perf_counter()
        jax.block_until_ready(fN(*devN))
        tNs.append(time.perf_counter() - t0)
    w1, wN = min(t1s), min(tNs)
    per_iter = (wN - w1) / (nrep - 1)
    _log("t1 ms: " + " ".join(f"{t*1e3:.2f}" for t in t1s))
    _log(f"t{nrep} ms: " + " ".join(f"{t*1e3:.2f}" for t in tNs))
    _log(f"measure: w1={w1*1e3:.3f}ms w{nrep}={wN*1e3:.3f}ms -> "
         f"{per_iter*1e6:.1f}us/iter")
    return per_iter * 1e9


_LAST_TIMING = None


def kernel(**inputs) -> np.ndarray:
    t0 = time.perf_counter()
    in_maps, F = prep_inputs(inputs)
    _log(f"host prep {time.perf_counter()-t0:.2f}s")
    t0 = time.perf_counter()
    nc = build_program(F)
    _log(f"build+tile {time.perf_counter()-t0:.1f}s")
    results, _, _ = run_spmd(nc, in_maps)
    out = np.empty((F, B, 64, 64), np.float32)
    for c in range(NCORES):
        yT = results[c]["yT"]                      # [4096, F*32]
        y = yT.T.reshape(F, BL, 64, 64)
        out[:, c * BL:(c + 1) * BL] = y
    return out

